# revision 13
# baseline (speedup 1.0000x reference)
"""Trainium2 Bass kernel for batch-8 multi-head attention.

Strategy: pure data parallelism — one batch element per NeuronCore (B=8,
8 cores), zero collectives.  All inputs are pre-arranged on the host so the
device kernel only ever runs dense matmuls in its preferred layouts:

  per-core DRAM inputs (bf16 unless noted):
    xT     [768, 1024]   x[b].T                    (feature-major activations)
    w_qk   [768, 1536]   [W_q * SCALE | W_k]       (stationary for QK^T)
    w_v    [768, 768]    W_v                       (moving for V)
    w_proj [768, 768]    W_proj                    (stationary for proj)
    b_all  [128, 18] f32 per-partition bias chunks (12 qk + 6 proj)
    b_v    [128, 12, 64] f32  V bias broadcast along partitions
  output:
    outT   [768, 1024] f32  (x[b] @ ... final)^T — host transposes back

Device pipeline per core (program order interleaves phases so ScalarE's exp
stream — the critical path — starts early and never starves):
  QK^T = w_qk^T @ xT                 -> 12 tiles [128, 1024], 2 heads/tile
  V    = xT^T @ w_v + b_v            -> 8 tiles [128, 12, 65], ones col fused
  per head h:
    S^T[m]   = K_h @ Q_h^T           (K=64 contraction, 2 heads / PE row-half)
    expS     = exp(S^T)  on ScalarE  (no max subtraction: |logits| < ~8)
    [O^T|s]  = [V_m|1]^T @ expS      (PSUM accumulate over m; row 64 = sums)
    oT       = O^T * (1/s)           (recip_approx + partition-broadcast + mul)
  outT = w_proj^T @ oT + b_proj

All PSUM tiles share two tags ("big" 2-bank x2, "o_ps" 1-bank x4 = 8 banks)
so phases hand PSUM off tile-by-tile with no pool barrier.
"""

import os
import sys

os.environ.setdefault("BASS_PERFETTO_PROFILE_ALL_CORES", "1")
if "/opt/trn_rl_repo" not in sys.path:
    sys.path.insert(0, "/opt/trn_rl_repo")

import numpy as np
import ml_dtypes

B, N, C, H = 8, 1024, 768, 12
D = C // H                # 64 head dim
SCALE = D ** -0.5
NCORES = 8
KT = C // 128             # 6 contraction tiles over C
MT = N // 128             # 8 token blocks
NJ = N // 512             # 2 query chunks of 512
BF16 = ml_dtypes.bfloat16

_CACHE = {}


def _enable_ldw_opt():
    """Flip walrus --enable-ldw-opt (dedup of back-to-back same-stationary
    LDWEIGHTS). Verified against the reference; revert if rel err moves."""
    from concourse import bass_utils as _bu
    if getattr(_bu, "_ldw_patched", False):
        return
    _orig = _bu.run_command

    def _patched(cmd, *a, **kw):
        cmd = ["--enable-ldw-opt=true" if c == "--enable-ldw-opt=false" else c
               for c in cmd]
        return _orig(cmd, *a, **kw)

    _bu.run_command = _patched
    _bu._ldw_patched = True


def build_nc():
    """Build + compile the per-core Bass graph (identical on all 8 cores)."""
    import concourse.tile as tile
    from concourse import bacc, mybir

    f32 = mybir.dt.float32
    bf16 = mybir.dt.bfloat16
    Exp = mybir.ActivationFunctionType.Exp

    # note: _enable_ldw_opt() crashes walrus codegen on this BIR — left off
    nc = bacc.Bacc("TRN2", target_bir_lowering=False, debug=False,
                   num_devices=NCORES)

    xT_e = nc.dram_tensor("xT", [C, N], bf16, kind="ExternalInput").ap()
    wqk_e = nc.dram_tensor("w_qk", [C, 2 * C], bf16, kind="ExternalInput").ap()
    wqk0_e = nc.dram_tensor("w_qk0", [C, 256], bf16, kind="ExternalInput").ap()
    wv_e = nc.dram_tensor("w_v", [C, C], bf16, kind="ExternalInput").ap()
    wp_e = nc.dram_tensor("w_proj", [C, C], bf16, kind="ExternalInput").ap()
    ball_e = nc.dram_tensor("b_all", [128, 18], f32, kind="ExternalInput").ap()
    bv_e = nc.dram_tensor("b_v", [128, H, D], f32, kind="ExternalInput").ap()
    out_e = nc.dram_tensor("outT", [C, N], f32, kind="ExternalOutput").ap()

    with tile.TileContext(nc) as tc:
        from contextlib import ExitStack

        with ExitStack() as es:
            persist = es.enter_context(tc.tile_pool(name="persist", bufs=1))
            s_pool = es.enter_context(tc.tile_pool(name="spsum", bufs=3, space="PSUM"))
            o_pool = es.enter_context(tc.tile_pool(name="opsum", bufs=2, space="PSUM"))
            e_pool = es.enter_context(tc.tile_pool(name="expS", bufs=14))
            r_pool = es.enter_context(tc.tile_pool(name="recip", bufs=2))
            rb_pool = es.enter_context(tc.tile_pool(name="recipb", bufs=2))
            out_pool = es.enter_context(tc.tile_pool(name="outc", bufs=3))

            # ---- persistent SBUF tiles ----------------------------------
            xT = [persist.tile([128, N], bf16, name=f"xT{k}", tag=f"xT{k}")
                  for k in range(KT)]
            wqk = [persist.tile([128, 2 * C], bf16, name=f"wqk{k}", tag=f"wqk{k}")
                   for k in range(KT)]
            wv = [persist.tile([128, C], bf16, name=f"wv{k}", tag=f"wv{k}")
                  for k in range(KT)]
            wp = [persist.tile([128, C], bf16, name=f"wp{k}", tag=f"wp{k}")
                  for k in range(KT)]
            wqk0 = [persist.tile([128, 256], bf16, name=f"wqk0_{k}", tag=f"wqk0_{k}")
                    for k in range(KT)]
            ball = persist.tile([128, 18], f32, name="ball", tag="ball")
            bv = persist.tile([128, H, D], f32, name="bv", tag="bv")
            qkT = [persist.tile([128, N], bf16, name=f"qkT{m}", tag=f"qkT{m}")
                   for m in range(12)]
            v_sb = [persist.tile([128, H, D + 1], bf16, name=f"v{t}", tag=f"v{t}")
                    for t in range(MT)]
            oT = [persist.tile([128, N], bf16, name=f"oT{m}", tag=f"oT{m}")
                  for m in range(KT)]

            # ---- input DMAs, in consumption order -----------------------
            nc.sync.dma_start(ball[:], ball_e[:])
            for k in range(KT):
                sl = slice(128 * k, 128 * (k + 1))
                nc.sync.dma_start(xT[k][:], xT_e[sl, :])
                nc.sync.dma_start(wqk0[k][:], wqk0_e[sl, :])
            for k in range(KT):
                sl = slice(128 * k, 128 * (k + 1))
                nc.sync.dma_start(wqk[k][:], wqk_e[sl, :])
            for k in range(KT):
                sl = slice(128 * k, 128 * (k + 1))
                nc.gpsimd.dma_start(wv[k][:], wv_e[sl, :])
            nc.gpsimd.dma_start(bv[:], bv_e[:])
            for k in range(KT):
                sl = slice(128 * k, 128 * (k + 1))
                nc.sync.dma_start(wp[k][:], wp_e[sl, :])

            # ---- filler sub-chunks (1-bank PSUM each, ~6 MMs) -----------
            qk_emitted = set()

            def sub_qk(m, w0=False):
                def emit():
                    qk_emitted.add(m)
                    ps = s_pool.tile([128, N], f32, name="sps", tag="sps")
                    for j in range(NJ):
                        for k in range(KT):
                            if w0:
                                lhsT = wqk0[k][:, 128 * (m // 6):128 * (m // 6 + 1)]
                            else:
                                lhsT = wqk[k][:, 128 * m:128 * (m + 1)]
                            nc.tensor.matmul(
                                ps[:, 512 * j:512 * (j + 1)],
                                lhsT=lhsT,
                                rhs=xT[k][:, 512 * j:512 * (j + 1)],
                                start=(k == 0), stop=(k == KT - 1),
                            )
                    nc.vector.tensor_scalar_add(qkT[m][:], ps[:], ball[:, m:m + 1])
                return emit

            def sub_v(t):
                def emit():
                    ps = s_pool.tile([128, N], f32, name="sps", tag="sps")
                    for c0, cw in ((0, 512), (512, 256)):
                        for k in range(KT):
                            nc.tensor.matmul(
                                ps[:, c0:c0 + cw],
                                lhsT=xT[k][:, 128 * t:128 * (t + 1)],
                                rhs=wv[k][:, c0:c0 + cw],
                                start=(k == 0), stop=(k == KT - 1),
                            )
                    nc.gpsimd.memset(v_sb[t][:, :, D:D + 1], 1.0)
                    nc.vector.tensor_add(
                        v_sb[t][:, :, 0:D],
                        ps[:, 0:C].rearrange("p (h x) -> p h x", x=D),
                        bv[:],
                    )
                return emit

            ph3a_out = [persist.tile([128, N], f32, name=f"p3a_{c}",
                                     tag=f"p3a_{c}") for c in range(KT)]

            def sub_ph3a(c):
                def emit():
                    ps = s_pool.tile([128, N], f32, name="sps", tag="sps")
                    for j in range(NJ):
                        for m in range(4):
                            nc.tensor.matmul(
                                ps[:, 512 * j:512 * (j + 1)],
                                lhsT=wp[m][:, 128 * c:128 * (c + 1)],
                                rhs=oT[m][:, 512 * j:512 * (j + 1)],
                                start=(m == 0), stop=(m == 3),
                            )
                    nc.vector.tensor_scalar_add(
                        ph3a_out[c][:], ps[:], ball[:, 12 + c:13 + c])
                return emit

            # ---- head-pair machinery ------------------------------------
            Exp_ = Exp

            def s_step(pair, m, e_e, e_o):
                """4 S matmuls alternating row-halves + 2 exps."""
                qt, kt = qkT[pair], qkT[6 + pair]
                t_e = s_pool.tile([128, N], f32, name="sps", tag="sps")
                t_o = s_pool.tile([128, N], f32, name="sps", tag="sps")
                for j in range(NJ):
                    for po, t in ((0, t_e), (64, t_o)):
                        nc.tensor.matmul(
                            t[:, 512 * j:512 * (j + 1)],
                            lhsT=kt[po:po + 64, 128 * m:128 * (m + 1)],
                            rhs=qt[po:po + 64, 512 * j:512 * (j + 1)],
                            start=True, stop=True,
                        )
                for t, lst in ((t_e, e_e), (t_o, e_o)):
                    e_sb = e_pool.tile([128, N], bf16, name="e_sb", tag="e_sb")
                    nc.scalar.activation(e_sb[:], t[:], Exp_)
                    lst.append(e_sb)

            class HeadPV:
                """Trailing PV + normalize for one head, consumed task-wise."""
                def __init__(self, h, e_tiles):
                    self.h, self.e = h, e_tiles
                    self.m = 0
                    self.o_ps = [o_pool.tile([65, 512], f32, name="o_ps",
                                             tag="o_ps") for _ in range(NJ)]

                def step(self):
                    h, m = self.h, self.m
                    for j in range(NJ):
                        nc.tensor.matmul(
                            self.o_ps[j][:, :],
                            lhsT=v_sb[m][:, h, :],
                            rhs=self.e[m][:, 512 * j:512 * (j + 1)],
                            start=(m == 0), stop=(m == MT - 1),
                        )
                    self.m += 1
                    if self.m == MT:
                        self.finish()
                        return True
                    return False

                def finish(self):
                    h, po = self.h, 64 * (self.h % 2)
                    sc = r_pool.tile([1, N], f32, name="sc", tag="sc")
                    for j in range(NJ):
                        nc.vector.tensor_copy(
                            sc[0:1, 512 * j:512 * (j + 1)], self.o_ps[j][64:65, :])
                    r = r_pool.tile([1, N], f32, name="r", tag="r")
                    nc.vector.reciprocal_approx_fast(r[0:1, :], sc[0:1, :])
                    rb = rb_pool.tile([64, N], f32, name="rb", tag="rb")
                    nc.gpsimd.partition_broadcast(rb[:], r[0:1, :])
                    for j in range(NJ):
                        nc.vector.tensor_mul(
                            oT[h // 2][po:po + 64, 512 * j:512 * (j + 1)],
                            self.o_ps[j][0:64, :],
                            rb[0:64, 512 * j:512 * (j + 1)],
                        )

            # ---- the software-pipelined schedule ------------------------
            from collections import deque
            fillers = deque()
            pv_queue = deque()   # HeadPV objects, strictly ordered

            def drain_pv(max_tasks):
                n = 0
                while pv_queue and n < max_tasks:
                    hp = pv_queue[0]
                    if hp.m >= len(hp.e):
                        break  # exp for this m not emitted yet
                    if hp.step():
                        pv_queue.popleft()
                    n += 1

            def drain_fillers(max_chunks):
                for _ in range(min(max_chunks, len(fillers))):
                    fillers.popleft()()

            # prelude: QK chunks for pair 0 (PE warmup, un-gated) — from the
            # small fast-loading wqk0 tensor so compute starts ~7us in
            sub_qk(0, w0=True)()
            sub_qk(6, w0=True)()
            # fillers for pair 0: all V blocks, then pair-1 QK chunks
            for t in range(MT):
                fillers.append(sub_v(t))
            for m in (1, 7):
                fillers.append(sub_qk(m))

            ph3a_pend = [sub_ph3a(c) for c in range(KT)]

            for pair in range(6):
                e_e, e_o = [], []
                pend_e, pend_o = HeadPV(2 * pair, e_e), HeadPV(2 * pair + 1, e_o)
                assert pair in (0,) or (pair in qk_emitted and
                                         6 + pair in qk_emitted), \
                    f"qk chunks for pair {pair} not emitted yet"
                new_fill = ([sub_qk(pair + 2), sub_qk(6 + pair + 2)]
                            if pair < 4 else [])
                for m in range(MT):
                    s_step(pair, m, e_e, e_o)
                    if m == 1:
                        pv_queue.append(pend_e)
                        pv_queue.append(pend_o)
                    drain_pv(3 if m <= 1 else 2)
                    if pair == 0:
                        drain_fillers(2 if m <= 4 else 1)
                    elif m % 2 == 0:
                        drain_fillers(1)
                    if new_fill and m == 1:
                        fillers.append(new_fill.pop(0))
                        fillers.append(new_fill.pop(0))
                    # once pairs <=3 fully retired, feed ph3 partials (m=0..3)
                    # as fillers for the otherwise filler-less pairs 4/5
                    if (pair >= 4 and ph3a_pend
                            and not any(hp.h <= 7 for hp in pv_queue)):
                        fillers.append(ph3a_pend.pop(0))
            # drain what remains
            while pv_queue:
                drain_pv(4)
                drain_fillers(1)
            drain_fillers(len(fillers))
            for fn in ph3a_pend:
                fn()

            # ---- phase 3 tail: add the m=4..5 contribution + DMA out ----
            for c in range(KT):
                ps = s_pool.tile([128, N], f32, name="sps", tag="sps")
                for j in range(NJ):
                    for mi, m in enumerate((4, 5)):
                        nc.tensor.matmul(
                            ps[:, 512 * j:512 * (j + 1)],
                            lhsT=wp[m][:, 128 * c:128 * (c + 1)],
                            rhs=oT[m][:, 512 * j:512 * (j + 1)],
                            start=(mi == 0), stop=(mi == 1),
                        )
                oc = out_pool.tile([128, N], f32, name="oc", tag="oc")
                nc.vector.tensor_add(oc[:], ps[:], ph3a_out[c][:])
                nc.sync.dma_start(out_e[128 * c:128 * (c + 1), :], oc[:])

    nc.compile()
    return nc


def prep_inputs(x, W_qkv, b_qkv, W_proj, b_proj):
    """Host-side shard + layout prep. Returns in_maps for 8 cores."""
    x = np.asarray(x, dtype=np.float32)
    W_qkv = np.asarray(W_qkv, dtype=np.float32)
    b_qkv = np.asarray(b_qkv, dtype=np.float32)
    W_proj = np.asarray(W_proj, dtype=np.float32)
    b_proj = np.asarray(b_proj, dtype=np.float32)

    w_qk = np.concatenate([W_qkv[:, :C] * SCALE, W_qkv[:, C:2 * C]], axis=1)
    w_qk = np.ascontiguousarray(w_qk).astype(BF16)
    w_v = np.ascontiguousarray(W_qkv[:, 2 * C:]).astype(BF16)
    w_p = W_proj.astype(BF16)

    b_qk = np.concatenate([b_qkv[:C] * SCALE, b_qkv[C:2 * C]])
    b_all = np.empty((128, 18), np.float32)
    b_all[:, :12] = b_qk.reshape(12, 128).T
    b_all[:, 12:] = b_proj.reshape(6, 128).T
    b_v = np.ascontiguousarray(
        np.broadcast_to(b_qkv[2 * C:].reshape(H, D), (128, H, D))).astype(np.float32)

    w_qk0 = np.ascontiguousarray(
        np.concatenate([w_qk[:, 0:128], w_qk[:, 768:896]], axis=1))
    shared = {"w_qk": w_qk, "w_qk0": w_qk0, "w_v": w_v, "w_proj": w_p,
              "b_all": b_all, "b_v": b_v}
    in_maps = []
    for b in range(NCORES):
        xT = np.ascontiguousarray(x[b].T).astype(BF16)
        m = dict(shared)
        m["xT"] = xT
        in_maps.append(m)
    return in_maps


def kernel(x, W_qkv, b_qkv, W_proj, b_proj):
    from concourse.bass_utils import run_bass_kernel_spmd

    nc = _CACHE.get("nc")
    if nc is None:
        nc = _CACHE["nc"] = build_nc()

    in_maps = prep_inputs(x, W_qkv, b_qkv, W_proj, b_proj)
    res = run_bass_kernel_spmd(nc, in_maps, core_ids=list(range(NCORES)))
    out = np.empty((B, N, C), np.float32)
    for b in range(NCORES):
        out[b] = res.results[b]["outT"].T
    return out


# revision 14
# speedup vs baseline: 1.2924x; 1.2924x over previous
"""Trainium2 Bass kernel for batch-8 multi-head attention.

Strategy: pure data parallelism — one batch element per NeuronCore (B=8,
8 cores), zero collectives.  All inputs are pre-arranged on the host so the
device kernel only ever runs dense matmuls in its preferred layouts:

  per-core DRAM inputs (bf16 unless noted):
    xT     [768, 1024]   x[b].T                    (feature-major activations)
    w_qk   [768, 1536]   [W_q * SCALE | W_k]       (stationary for QK^T)
    w_v    [768, 768]    W_v                       (moving for V)
    w_proj [768, 768]    W_proj                    (stationary for proj)
    b_all  [128, 18] f32 per-partition bias chunks (12 qk + 6 proj)
    b_v    [128, 12, 64] f32  V bias broadcast along partitions
  output:
    outT   [768, 1024] f32  (x[b] @ ... final)^T — host transposes back

Device pipeline per core (program order interleaves phases so ScalarE's exp
stream — the critical path — starts early and never starves):
  QK^T = w_qk^T @ xT                 -> 12 tiles [128, 1024], 2 heads/tile
  V    = xT^T @ w_v + b_v            -> 8 tiles [128, 12, 65], ones col fused
  per head h:
    S^T[m]   = K_h @ Q_h^T           (K=64 contraction, 2 heads / PE row-half)
    expS     = exp(S^T)  on ScalarE  (no max subtraction: |logits| < ~8)
    [O^T|s]  = [V_m|1]^T @ expS      (PSUM accumulate over m; row 64 = sums)
    oT       = O^T * (1/s)           (recip_approx + partition-broadcast + mul)
  outT = w_proj^T @ oT + b_proj

All PSUM tiles share two tags ("big" 2-bank x2, "o_ps" 1-bank x4 = 8 banks)
so phases hand PSUM off tile-by-tile with no pool barrier.
"""

import os
import sys

os.environ.setdefault("BASS_PERFETTO_PROFILE_ALL_CORES", "1")
if "/opt/trn_rl_repo" not in sys.path:
    sys.path.insert(0, "/opt/trn_rl_repo")

import numpy as np
import ml_dtypes

B, N, C, H = 8, 1024, 768, 12
D = C // H                # 64 head dim
SCALE = D ** -0.5
NCORES = 8
KT = C // 128             # 6 contraction tiles over C
MT = N // 128             # 8 token blocks
NJ = N // 512             # 2 query chunks of 512
BF16 = ml_dtypes.bfloat16

_CACHE = {}


def _enable_ldw_opt():
    """Flip walrus --enable-ldw-opt (dedup of back-to-back same-stationary
    LDWEIGHTS). Verified against the reference; revert if rel err moves."""
    from concourse import bass_utils as _bu
    if getattr(_bu, "_ldw_patched", False):
        return
    _orig = _bu.run_command

    def _patched(cmd, *a, **kw):
        cmd = ["--enable-ldw-opt=true" if c == "--enable-ldw-opt=false" else c
               for c in cmd]
        return _orig(cmd, *a, **kw)

    _bu.run_command = _patched
    _bu._ldw_patched = True


def build_nc():
    """Build + compile the per-core Bass graph (identical on all 8 cores)."""
    import concourse.tile as tile
    from concourse import bacc, mybir

    f32 = mybir.dt.float32
    bf16 = mybir.dt.bfloat16
    Exp = mybir.ActivationFunctionType.Exp

    # note: _enable_ldw_opt() crashes walrus codegen on this BIR — left off
    nc = bacc.Bacc("TRN2", target_bir_lowering=False, debug=False,
                   num_devices=NCORES)

    xT_e = nc.dram_tensor("xT", [C, N], bf16, kind="ExternalInput").ap()
    wqk_e = nc.dram_tensor("w_qk", [C, 2 * C], bf16, kind="ExternalInput").ap()
    wqk0_e = nc.dram_tensor("w_qk0", [C, 256], bf16, kind="ExternalInput").ap()
    wv_e = nc.dram_tensor("w_v", [C, C], bf16, kind="ExternalInput").ap()
    wp_e = nc.dram_tensor("w_proj", [C, C], bf16, kind="ExternalInput").ap()
    ball_e = nc.dram_tensor("b_all", [128, 18], f32, kind="ExternalInput").ap()
    bv_e = nc.dram_tensor("b_v", [128, H, D], f32, kind="ExternalInput").ap()
    out_e = nc.dram_tensor("outT", [C, N], f32, kind="ExternalOutput").ap()

    with tile.TileContext(nc) as tc:
        from contextlib import ExitStack

        with ExitStack() as es:
            persist = es.enter_context(tc.tile_pool(name="persist", bufs=1))
            s_pool = es.enter_context(tc.tile_pool(name="spsum", bufs=2, space="PSUM"))
            o_pool = es.enter_context(tc.tile_pool(name="opsum", bufs=2, space="PSUM"))
            f_pool = es.enter_context(tc.tile_pool(name="fpsum", bufs=2, space="PSUM"))
            e_pool = es.enter_context(tc.tile_pool(name="expS", bufs=14))
            r_pool = es.enter_context(tc.tile_pool(name="recip", bufs=2))
            rb_pool = es.enter_context(tc.tile_pool(name="recipb", bufs=2))
            out_pool = es.enter_context(tc.tile_pool(name="outc", bufs=3))

            # ---- persistent SBUF tiles ----------------------------------
            xT = [persist.tile([128, N], bf16, name=f"xT{k}", tag=f"xT{k}")
                  for k in range(KT)]
            wqk = [persist.tile([128, 2 * C], bf16, name=f"wqk{k}", tag=f"wqk{k}")
                   for k in range(KT)]
            wv = [persist.tile([128, C], bf16, name=f"wv{k}", tag=f"wv{k}")
                  for k in range(KT)]
            wp = [persist.tile([128, C], bf16, name=f"wp{k}", tag=f"wp{k}")
                  for k in range(KT)]
            wqk0 = [persist.tile([128, 256], bf16, name=f"wqk0_{k}", tag=f"wqk0_{k}")
                    for k in range(KT)]
            ball = persist.tile([128, 18], f32, name="ball", tag="ball")
            bv = persist.tile([128, H, D], f32, name="bv", tag="bv")
            qkT = [persist.tile([128, N], bf16, name=f"qkT{m}", tag=f"qkT{m}")
                   for m in range(12)]
            v_sb = [persist.tile([128, H, D + 1], bf16, name=f"v{t}", tag=f"v{t}")
                    for t in range(MT)]
            oT = [persist.tile([128, N], bf16, name=f"oT{m}", tag=f"oT{m}")
                  for m in range(KT)]

            # ---- input DMAs, in consumption order -----------------------
            nc.sync.dma_start(ball[:], ball_e[:])
            for k in range(KT):
                sl = slice(128 * k, 128 * (k + 1))
                nc.sync.dma_start(xT[k][:], xT_e[sl, :])
                nc.sync.dma_start(wqk0[k][:], wqk0_e[sl, :])
            for k in range(KT):
                sl = slice(128 * k, 128 * (k + 1))
                nc.sync.dma_start(wqk[k][:], wqk_e[sl, :])
            for k in range(KT):
                sl = slice(128 * k, 128 * (k + 1))
                nc.gpsimd.dma_start(wv[k][:], wv_e[sl, :])
            nc.gpsimd.dma_start(bv[:], bv_e[:])
            for k in range(KT):
                sl = slice(128 * k, 128 * (k + 1))
                nc.sync.dma_start(wp[k][:], wp_e[sl, :])

            # ---- filler sub-chunks (1-bank PSUM each, ~6 MMs) -----------
            def sub_qk(m, j, w0=False):
                def emit():
                    ps = f_pool.tile([128, 512], f32, name="fps", tag="fps")
                    for k in range(KT):
                        if w0:
                            lhsT = wqk0[k][:, 128 * (m // 6):128 * (m // 6 + 1)]
                        else:
                            lhsT = wqk[k][:, 128 * m:128 * (m + 1)]
                        nc.tensor.matmul(
                            ps[:],
                            lhsT=lhsT,
                            rhs=xT[k][:, 512 * j:512 * (j + 1)],
                            start=(k == 0), stop=(k == KT - 1),
                        )
                    nc.vector.tensor_scalar_add(
                        qkT[m][:, 512 * j:512 * (j + 1)], ps[:], ball[:, m:m + 1])
                return emit

            def sub_v(t, part):
                c0, cw = ((0, 512), (512, 256))[part]
                h0, hn = ((0, 8), (8, 4))[part]
                def emit():
                    ps = f_pool.tile([128, 512], f32, name="fps", tag="fps")
                    for k in range(KT):
                        nc.tensor.matmul(
                            ps[:, 0:cw],
                            lhsT=xT[k][:, 128 * t:128 * (t + 1)],
                            rhs=wv[k][:, c0:c0 + cw],
                            start=(k == 0), stop=(k == KT - 1),
                        )
                    if part == 0:
                        nc.gpsimd.memset(v_sb[t][:, :, D:D + 1], 1.0)
                    nc.vector.tensor_add(
                        v_sb[t][:, h0:h0 + hn, 0:D],
                        ps[:, 0:cw].rearrange("p (h x) -> p h x", x=D),
                        bv[:, h0:h0 + hn, :],
                    )
                return emit

            ph3a_out = [[persist.tile([128, 512], f32, name=f"p3a_{c}_{j}",
                                      tag=f"p3a_{c}_{j}") for j in range(NJ)]
                        for c in range(KT)]

            def sub_ph3a(c, j):
                def emit():
                    ps = f_pool.tile([128, 512], f32, name="fps", tag="fps")
                    for m in range(4):
                        nc.tensor.matmul(
                            ps[:],
                            lhsT=wp[m][:, 128 * c:128 * (c + 1)],
                            rhs=oT[m][:, 512 * j:512 * (j + 1)],
                            start=(m == 0), stop=(m == 3),
                        )
                    nc.vector.tensor_scalar_add(
                        ph3a_out[c][j][:], ps[:], ball[:, 12 + c:13 + c])
                return emit

            # ---- head-pair machinery ------------------------------------
            Exp_ = Exp

            def s_step(pair, m, e_e, e_o):
                """4 S matmuls alternating row-halves + 2 exps."""
                qt, kt = qkT[pair], qkT[6 + pair]
                t_e = s_pool.tile([128, N], f32, name="sps", tag="sps")
                t_o = s_pool.tile([128, N], f32, name="sps", tag="sps")
                for j in range(NJ):
                    for po, t in ((0, t_e), (64, t_o)):
                        nc.tensor.matmul(
                            t[:, 512 * j:512 * (j + 1)],
                            lhsT=kt[po:po + 64, 128 * m:128 * (m + 1)],
                            rhs=qt[po:po + 64, 512 * j:512 * (j + 1)],
                            start=True, stop=True,
                        )
                for t, lst in ((t_e, e_e), (t_o, e_o)):
                    e_sb = e_pool.tile([128, N], bf16, name="e_sb", tag="e_sb")
                    nc.scalar.activation(e_sb[:], t[:], Exp_)
                    lst.append(e_sb)

            class HeadPV:
                """Trailing PV + normalize for one head, consumed task-wise."""
                def __init__(self, h, e_tiles):
                    self.h, self.e = h, e_tiles
                    self.m = 0
                    self.o_ps = [o_pool.tile([65, 512], f32, name="o_ps",
                                             tag="o_ps") for _ in range(NJ)]

                def step(self):
                    h, m = self.h, self.m
                    for j in range(NJ):
                        nc.tensor.matmul(
                            self.o_ps[j][:, :],
                            lhsT=v_sb[m][:, h, :],
                            rhs=self.e[m][:, 512 * j:512 * (j + 1)],
                            start=(m == 0), stop=(m == MT - 1),
                        )
                    self.m += 1
                    if self.m == MT:
                        self.finish()
                        return True
                    return False

                def finish(self):
                    h, po = self.h, 64 * (self.h % 2)
                    sc = r_pool.tile([1, N], f32, name="sc", tag="sc")
                    for j in range(NJ):
                        nc.vector.tensor_copy(
                            sc[0:1, 512 * j:512 * (j + 1)], self.o_ps[j][64:65, :])
                    r = r_pool.tile([1, N], f32, name="r", tag="r")
                    nc.vector.reciprocal_approx_fast(r[0:1, :], sc[0:1, :])
                    rb = rb_pool.tile([64, N], f32, name="rb", tag="rb")
                    nc.gpsimd.partition_broadcast(rb[:], r[0:1, :])
                    for j in range(NJ):
                        nc.vector.tensor_mul(
                            oT[h // 2][po:po + 64, 512 * j:512 * (j + 1)],
                            self.o_ps[j][0:64, :],
                            rb[0:64, 512 * j:512 * (j + 1)],
                        )

            # ---- the software-pipelined schedule ------------------------
            from collections import deque
            fillers = deque()
            pv_queue = deque()   # HeadPV objects, strictly ordered

            def drain_pv(max_tasks):
                n = 0
                while pv_queue and n < max_tasks:
                    hp = pv_queue[0]
                    if hp.m >= len(hp.e):
                        break  # exp for this m not emitted yet
                    if hp.step():
                        pv_queue.popleft()
                    n += 1

            def drain_fillers(max_chunks):
                for _ in range(min(max_chunks, len(fillers))):
                    fillers.popleft()()

            # prelude: QK chunks for pair 0 (PE warmup, un-gated) — from the
            # small fast-loading wqk0 tensor so compute starts ~7us in
            for j in range(NJ):
                sub_qk(0, j, w0=True)()
            for j in range(NJ):
                sub_qk(6, j, w0=True)()
            # fillers for pair 0: all V sub-chunks, then pair-1 QK chunks
            for t in range(MT):
                fillers.append(sub_v(t, 0))
                fillers.append(sub_v(t, 1))
            for m in (1, 7):
                for j in range(NJ):
                    fillers.append(sub_qk(m, j))

            ph3a_pend = [sub_ph3a(c, j) for c in range(KT) for j in range(NJ)]

            for pair in range(6):
                e_e, e_o = [], []
                pend_e, pend_o = HeadPV(2 * pair, e_e), HeadPV(2 * pair + 1, e_o)
                if pair < 5:
                    new_fill = [sub_qk(pair + 2, j) for j in range(NJ)] +                                [sub_qk(6 + pair + 2, j) for j in range(NJ)]                                if pair < 4 else []
                else:
                    new_fill = []
                for m in range(MT):
                    s_step(pair, m, e_e, e_o)
                    if m == 2:
                        pv_queue.append(pend_e)
                        pv_queue.append(pend_o)
                    drain_pv(2)
                    nfill = 3 if pair == 0 else 1
                    drain_fillers(nfill)
                    if new_fill and m % 4 == 1:
                        fillers.append(new_fill.pop(0))
                        fillers.append(new_fill.pop(0))
                    # once pairs <=3 fully retired, feed ph3 partials (m=0..3)
                    # as fillers for the otherwise filler-less pairs 4/5
                    if (pair >= 4 and ph3a_pend
                            and not any(hp.h <= 7 for hp in pv_queue)):
                        fillers.append(ph3a_pend.pop(0))
                        if ph3a_pend:
                            fillers.append(ph3a_pend.pop(0))
            # drain what remains
            while pv_queue:
                drain_pv(4)
                drain_fillers(1)
            drain_fillers(len(fillers))
            for fn in ph3a_pend:
                fn()

            # ---- phase 3 tail: add the m=4..5 contribution + DMA out ----
            for c in range(KT):
                for j in range(NJ):
                    ps = f_pool.tile([128, 512], f32, name="fps", tag="fps")
                    for mi, m in enumerate((4, 5)):
                        nc.tensor.matmul(
                            ps[:],
                            lhsT=wp[m][:, 128 * c:128 * (c + 1)],
                            rhs=oT[m][:, 512 * j:512 * (j + 1)],
                            start=(mi == 0), stop=(mi == 1),
                        )
                    oc = out_pool.tile([128, 512], f32, name="oc", tag="oc")
                    nc.vector.tensor_add(oc[:], ps[:], ph3a_out[c][j][:])
                    nc.sync.dma_start(
                        out_e[128 * c:128 * (c + 1), 512 * j:512 * (j + 1)], oc[:])

    nc.compile()
    return nc


def prep_inputs(x, W_qkv, b_qkv, W_proj, b_proj):
    """Host-side shard + layout prep. Returns in_maps for 8 cores."""
    x = np.asarray(x, dtype=np.float32)
    W_qkv = np.asarray(W_qkv, dtype=np.float32)
    b_qkv = np.asarray(b_qkv, dtype=np.float32)
    W_proj = np.asarray(W_proj, dtype=np.float32)
    b_proj = np.asarray(b_proj, dtype=np.float32)

    w_qk = np.concatenate([W_qkv[:, :C] * SCALE, W_qkv[:, C:2 * C]], axis=1)
    w_qk = np.ascontiguousarray(w_qk).astype(BF16)
    w_v = np.ascontiguousarray(W_qkv[:, 2 * C:]).astype(BF16)
    w_p = W_proj.astype(BF16)

    b_qk = np.concatenate([b_qkv[:C] * SCALE, b_qkv[C:2 * C]])
    b_all = np.empty((128, 18), np.float32)
    b_all[:, :12] = b_qk.reshape(12, 128).T
    b_all[:, 12:] = b_proj.reshape(6, 128).T
    b_v = np.ascontiguousarray(
        np.broadcast_to(b_qkv[2 * C:].reshape(H, D), (128, H, D))).astype(np.float32)

    w_qk0 = np.ascontiguousarray(
        np.concatenate([w_qk[:, 0:128], w_qk[:, 768:896]], axis=1))
    shared = {"w_qk": w_qk, "w_qk0": w_qk0, "w_v": w_v, "w_proj": w_p,
              "b_all": b_all, "b_v": b_v}
    in_maps = []
    for b in range(NCORES):
        xT = np.ascontiguousarray(x[b].T).astype(BF16)
        m = dict(shared)
        m["xT"] = xT
        in_maps.append(m)
    return in_maps


def kernel(x, W_qkv, b_qkv, W_proj, b_proj):
    from concourse.bass_utils import run_bass_kernel_spmd

    nc = _CACHE.get("nc")
    if nc is None:
        nc = _CACHE["nc"] = build_nc()

    in_maps = prep_inputs(x, W_qkv, b_qkv, W_proj, b_proj)
    res = run_bass_kernel_spmd(nc, in_maps, core_ids=list(range(NCORES)))
    out = np.empty((B, N, C), np.float32)
    for b in range(NCORES):
        out[b] = res.results[b]["outT"].T
    return out
